# revision 13
# baseline (speedup 1.0000x reference)
"""AttnBlock (GroupNorm + single-head spatial self-attention + residual) on 8 TRN2 cores.

Sharding: data-parallel over batch — B=16 images, 2 per NeuronCore. Each core runs
an identical Bass/Tile program over its 2 images; no cross-core communication.

Per-image pipeline (all on one core, C=512 channels, HW=1024 spatial):
  1. GroupNorm(32 groups), pipelined per 128-channel tile: per-channel sum/sumsq
     (DVE/ACT), group-combine via a tiny matmul with a 0/1 group selector,
     broadcast back via its transpose.
  2. q,k (C x HW, channel-partitioned) and vT (HW x C, spatial-partitioned)
     via 1x1-conv matmuls against pre-transposed weights.
  3. scores^T[j,i] = sum_c k[c,j] q[c,i]; exp (with the C^-0.5 scale folded into
     the ACT activation) -> P^T; den[i] = sum_j P^T via ones-matmul.
  4. num[c,i] = sum_j vT[j,c] P^T[j,i]; the 1/den softmax normalization is folded
     into num's PSUM eviction (it commutes with the channel-wise wo projection).
  5. proj = woT.T @ num; out = x + bo + proj.

The attention internals run in bf16 (matmul operands; fp32 PSUM accumulation);
GroupNorm stats and hn are computed from a bf16 copy of x. The residual path
(x, final add) stays fp32. Measured error vs the fp32 reference: ~3e-5 relative.
"""

import numpy as np
import ml_dtypes
from contextlib import ExitStack

import concourse.bass as bass
import concourse.bacc as bacc
import concourse.tile as tile
import concourse.mybir as mybir
from concourse.bass_utils import run_bass_kernel_spmd

F32 = mybir.dt.float32
AF = mybir.ActivationFunctionType
OP = mybir.AluOpType
AX = mybir.AxisListType

B, C, H, W = 16, 512, 32, 32
HW = H * W            # 1024
G = 32                # groupnorm groups
CPG = C // G          # 16 channels per group
EPS = 1e-5
NCORES = 8
BPC = B // NCORES     # 2 images per core
P = 128               # SBUF partitions
NCT = C // P          # 4 channel tiles
GPT = P // CPG        # 8 groups per channel tile
NSB = HW // P         # 8 spatial blocks of 128
FC = 512              # matmul moving-dim chunk (one PSUM bank of fp32)
NIC = HW // FC        # 2 chunks over the spatial free dim
SM_SCALE = float(C) ** -0.5

# Attention-internals dtype.
DT = mybir.dt.bfloat16
DT_NP = ml_dtypes.bfloat16

_CACHE: dict = {}


def _mm(nc, out, lhsT, rhs, start, stop):
    nc.tensor.matmul(out, lhsT, rhs, start=start, stop=stop)


def _emit(ctx, tc, io):
    nc = tc.nc
    dmae = [nc.sync, nc.scalar]  # the two HWDGE queues; alternate for bandwidth

    consts = ctx.enter_context(tc.tile_pool(name="consts", bufs=1))
    pX16 = ctx.enter_context(tc.tile_pool(name="pX16", bufs=2))
    pX = ctx.enter_context(tc.tile_pool(name="pX", bufs=2))
    pHN = ctx.enter_context(tc.tile_pool(name="pHN", bufs=2))
    pQ = ctx.enter_context(tc.tile_pool(name="pQ", bufs=1))
    pK = ctx.enter_context(tc.tile_pool(name="pK", bufs=1))
    pVT = ctx.enter_context(tc.tile_pool(name="pVT", bufs=1))
    pPT = ctx.enter_context(tc.tile_pool(name="pPT", bufs=1))
    pNUM = ctx.enter_context(tc.tile_pool(name="pNUM", bufs=1))
    pOUT = ctx.enter_context(tc.tile_pool(name="pOUT", bufs=2))
    pS = ctx.enter_context(tc.tile_pool(name="pS", bufs=2))
    pmm = ctx.enter_context(tc.tile_pool(name="pmm", bufs=4, space="PSUM"))
    paux = ctx.enter_context(tc.tile_pool(name="paux", bufs=2, space="PSUM"))
    ptiny = ctx.enter_context(tc.tile_pool(name="ptiny", bufs=2, space="PSUM"))

    # ---- tiny constants first (KBs; the group selector gates the first matmul) ----
    def load_const(name, shape, dtype=F32, e=0):
        t = consts.tile(list(shape), dtype, name=f"c_{name}")
        dmae[e].dma_start(t[:], io[name][:])
        return t

    gsel = load_const("gsel", (P, GPT))
    gselT = load_const("gselT", (GPT, P), e=1)
    gs_sb = load_const("gn_s", (P, NCT))
    gb_sb = load_const("gn_b", (P, NCT), e=1)
    bq_sb = load_const("bq_c", (P, NCT))
    bk_sb = load_const("bk_c", (P, NCT), e=1)
    bo_sb = load_const("bo_c", (P, NCT))
    bv_r = load_const("bv_r", (1, C), e=1)

    ones_col = consts.tile([P, 1], DT, name="ones_col")
    nc.vector.memset(ones_col[:], 1.0)
    ones_row = consts.tile([1, P], DT, name="ones_row")
    nc.vector.memset(ones_row[:], 1.0)
    zb = consts.tile([P, 1], F32, name="zb")
    nc.vector.memset(zb[:], 0.0)
    epsb = consts.tile([GPT, 1], F32, name="epsb")
    nc.vector.memset(epsb[:], EPS)

    w_sb = {}

    def emit_weights():
        # emitted after image 0's x16 DMAs so those win the queue race
        for ct in range(NCT):  # ct-major: qkv's first k-tiles land first
            for wi, wname in enumerate(("wqt", "wkt", "wvt", "wot")):
                t = w_sb.setdefault(wname, [None] * NCT)
                t[ct] = consts.tile([P, C], DT, name=f"{wname}{ct}")
                dmae[(ct * 4 + wi) % 2].dma_start(
                    t[ct][:], io[wname][ct * P:(ct + 1) * P, :])
        # bv broadcast to all partitions: ones_row.T @ bv_r  (K=1 matmul)
        bv_rdt = consts.tile([1, C], DT, name="bv_rdt")
        nc.vector.tensor_copy(bv_rdt[:], bv_r[:])
        bvb_ps = pmm.tile([P, C], F32, name="bvb_ps", tag="mm")
        _mm(nc, bvb_ps[:], ones_row[:], bv_rdt[:], start=True, stop=True)
        bv_b = consts.tile([P, C], F32, name="bv_b")
        nc.vector.tensor_copy(bv_b[:], bvb_ps[:])
        w_sb["bv_b"] = bv_b

    def emit_load16(im):
        i = im["i"]
        X16 = pX16.tile([P, NCT, HW], DT, name=f"X16_{i}", tag="X16")
        for ct in range(NCT):
            dmae[ct % 2].dma_start(X16[:, ct, :], io["x16"][i, ct * P:(ct + 1) * P, :])
        im["X16"] = X16

    def emit_load32(im):
        i = im["i"]
        X = pX.tile([P, NCT, HW], F32, name=f"X{i}", tag="X")
        for ct in range(NCT):
            dmae[ct % 2].dma_start(X[:, ct, :], io["x"][i, ct * P:(ct + 1) * P, :])
        im["X"] = X

    def emit_stats(im):
        i = im["i"]
        X16 = im["X16"]
        stats = pS.tile([P, NCT, 2], F32, name=f"stats{i}", tag="stats")
        scratch = pS.tile([P, HW], DT, name=f"scr{i}", tag="scratch")
        for ct in range(NCT):
            nc.vector.tensor_reduce(stats[:, ct, 0:1], X16[:, ct, :], AX.X, OP.add)
            nc.scalar.activation(scratch[:], X16[:, ct, :], AF.Square, bias=zb[:],
                                 accum_out=stats[:, ct, 1:2])
        im["stats"] = stats

    def emit_norm(im):
        # per-c-tile chain so hn[0] is ready before later tiles' stats complete
        i = im["i"]
        X16, stats = im["X16"], im["stats"]
        HN = pHN.tile([P, NCT, HW], DT, name=f"HN{i}", tag="HN")
        with nc.named_scope(f"norm{i}"):
            for ct in range(NCT):
                gst = ptiny.tile([GPT, 2], F32, name=f"gst{i}_{ct}", tag="tiny")
                _mm(nc, gst[:], gsel[:], stats[:, ct, :], start=True, stop=True)
                gm = pS.tile([GPT, 2], F32, name=f"gm{i}_{ct}", tag="gm")
                nc.vector.tensor_scalar_mul(gm[:], gst[:], 1.0 / (CPG * HW))
                sq = pS.tile([GPT, 1], F32, name=f"sq{i}_{ct}", tag="sq")
                nc.vector.tensor_mul(sq[:], gm[:, 0:1], gm[:, 0:1])
                var = pS.tile([GPT, 1], F32, name=f"var{i}_{ct}", tag="var")
                nc.vector.tensor_sub(var[:], gm[:, 1:2], sq[:])
                std = pS.tile([GPT, 1], F32, name=f"std{i}_{ct}", tag="std")
                nc.scalar.activation(std[:], var[:], AF.Sqrt, bias=epsb[:])
                # gmr = [-mean*rstd, rstd]
                gmr = pS.tile([GPT, 2], F32, name=f"gmr{i}_{ct}", tag="gmr")
                nc.vector.reciprocal(gmr[:, 1:2], std[:])
                nc.vector.scalar_tensor_tensor(gmr[:, 0:1], gm[:, 0:1], -1.0,
                                               gmr[:, 1:2], OP.mult, OP.mult)
                pmr = ptiny.tile([P, 2], F32, name=f"pmr{i}_{ct}", tag="tiny")
                _mm(nc, pmr[:], gselT[:], gmr[:], start=True, stop=True)
                # a = rstd*gn_scale, b = gn_bias - mean*rstd*gn_scale
                ab = pS.tile([P, 2], F32, name=f"ab{i}_{ct}", tag="ab")
                nc.vector.tensor_mul(ab[:, 1:2], pmr[:, 1:2], gs_sb[:, ct:ct + 1])
                nc.vector.scalar_tensor_tensor(ab[:, 0:1], pmr[:, 0:1],
                                               gs_sb[:, ct:ct + 1],
                                               gb_sb[:, ct:ct + 1], OP.mult, OP.add)
                nc.vector.tensor_scalar(HN[:, ct, :], X16[:, ct, :],
                                        ab[:, 1:2], ab[:, 0:1], OP.mult, OP.add)
        im["HN"] = HN

    def emit_qkv(im):
        i = im["i"]
        HN = im["HN"]
        with nc.named_scope(f"qkv{i}"):
            Q = pQ.tile([P, NCT, HW], DT, name=f"Q{i}", tag="Q")
            K = pK.tile([P, NCT, HW], DT, name=f"K{i}", tag="K")
            for wname, bias_sb, OT in (("wqt", bq_sb, Q), ("wkt", bk_sb, K)):
                for ob in range(NCT):
                    ps = [pmm.tile([P, FC], F32, name=f"{wname}ps{i}_{ob}_{ic}", tag="mm")
                          for ic in range(NIC)]
                    for ct in range(NCT):
                        lhs = w_sb[wname][ct][:, ob * P:(ob + 1) * P]
                        for ic in range(NIC):
                            _mm(nc, ps[ic][:], lhs, HN[:, ct, ic * FC:(ic + 1) * FC],
                                start=(ct == 0), stop=(ct == NCT - 1))
                    for ic in range(NIC):
                        nc.scalar.add(OT[:, ob, ic * FC:(ic + 1) * FC], ps[ic][:],
                                      bias_sb[:, ob:ob + 1])
            VT = pVT.tile([P, NSB, C], DT, name=f"VT{i}", tag="VT")
            for sb in range(NSB):
                ps = pmm.tile([P, C], F32, name=f"vtps{i}_{sb}", tag="mm")
                for ct in range(NCT):
                    _mm(nc, ps[:], HN[:, ct, sb * P:(sb + 1) * P], w_sb["wvt"][ct][:],
                        start=(ct == 0), stop=(ct == NCT - 1))
                nc.vector.tensor_add(VT[:, sb, :], ps[:], w_sb["bv_b"][:])
            im["Q"], im["K"], im["VT"] = Q, K, VT

    def emit_scores(im):
        i = im["i"]
        Q, K = im["Q"], im["K"]
        with nc.named_scope(f"scores{i}"):
            PT = pPT.tile([P, NSB, HW], DT, name=f"PT{i}", tag="PT")
            for jb in range(NSB):
                ps = [pmm.tile([P, FC], F32, name=f"sps{i}_{jb}_{ic}", tag="mm")
                      for ic in range(NIC)]
                for ct in range(NCT):
                    lhs = K[:, ct, jb * P:(jb + 1) * P]
                    for ic in range(NIC):
                        _mm(nc, ps[ic][:], lhs, Q[:, ct, ic * FC:(ic + 1) * FC],
                            start=(ct == 0), stop=(ct == NCT - 1))
                for ic in range(NIC):
                    nc.scalar.activation(PT[:, jb, ic * FC:(ic + 1) * FC], ps[ic][:],
                                         AF.Exp, bias=zb[:], scale=SM_SCALE)
            recip = pS.tile([1, HW], F32, name=f"recip{i}", tag="recip")
            recip_dt = pS.tile([1, HW], DT, name=f"recipdt{i}", tag="recipdt")
            for ic in range(NIC):
                den = paux.tile([1, FC], F32, name=f"den{i}_{ic}", tag="aux")
                for jb in range(NSB):
                    _mm(nc, den[:], ones_col[:], PT[:, jb, ic * FC:(ic + 1) * FC],
                        start=(jb == 0), stop=(jb == NSB - 1))
                sl = slice(ic * FC, (ic + 1) * FC)
                nc.vector.reciprocal(recip[:, sl], den[:])
                nc.vector.tensor_copy(recip_dt[:, sl], recip[:, sl])
            im["recipdt"] = recip_dt
            im["PT"] = PT

    def emit_attn_out(im):
        i = im["i"]
        X, VT, PT = im["X"], im["VT"], im["PT"]
        with nc.named_scope(f"attnout{i}"):
            # broadcast 1/den to all partitions (needed by num evictions below)
            recipb = pS.tile([P, HW], F32, name=f"recipb{i}", tag="recipb")
            for ic in range(NIC):
                rb = paux.tile([P, FC], F32, name=f"rb{i}_{ic}", tag="aux")
                _mm(nc, rb[:], ones_row[:], im["recipdt"][:, ic * FC:(ic + 1) * FC],
                    start=True, stop=True)
                nc.vector.tensor_copy(recipb[:, ic * FC:(ic + 1) * FC], rb[:])
            # num = vT.T @ P^T, normalized by 1/den during eviction
            NUM = pNUM.tile([P, NCT, HW], DT, name=f"NUM{i}", tag="NUM")
            for cb in range(NCT):
                ps = [pmm.tile([P, FC], F32, name=f"nps{i}_{cb}_{ic}", tag="mm")
                      for ic in range(NIC)]
                for jt in range(NSB):
                    lhs = VT[:, jt, cb * P:(cb + 1) * P]
                    for ic in range(NIC):
                        _mm(nc, ps[ic][:], lhs, PT[:, jt, ic * FC:(ic + 1) * FC],
                            start=(jt == 0), stop=(jt == NSB - 1))
                for ic in range(NIC):
                    sl = slice(ic * FC, (ic + 1) * FC)
                    nc.vector.tensor_mul(NUM[:, cb, sl], ps[ic][:], recipb[:, sl])
            # proj + residual(+bo) straight from PSUM, then store
            OUTT = pOUT.tile([P, NCT, HW], F32, name=f"OUT{i}", tag="OUT")
            for ob in range(NCT):
                ps = [pmm.tile([P, FC], F32, name=f"pps{i}_{ob}_{ic}", tag="mm")
                      for ic in range(NIC)]
                for ct in range(NCT):
                    lhs = w_sb["wot"][ct][:, ob * P:(ob + 1) * P]
                    for ic in range(NIC):
                        _mm(nc, ps[ic][:], lhs, NUM[:, ct, ic * FC:(ic + 1) * FC],
                            start=(ct == 0), stop=(ct == NCT - 1))
                for ic in range(NIC):
                    sl = slice(ic * FC, (ic + 1) * FC)
                    nc.vector.scalar_tensor_tensor(OUTT[:, ob, sl], ps[ic][:],
                                                   bo_sb[:, ob:ob + 1], X[:, ob, sl],
                                                   OP.add, OP.add)
                    dmae[(ob + ic) % 2].dma_start(
                        io["out"][i, ob * P:(ob + 1) * P, sl], OUTT[:, ob, sl])

    ims = [{"i": i} for i in range(BPC)]
    a, b = ims
    emit_load16(a)
    emit_weights()
    emit_load16(b)
    emit_stats(a)
    emit_norm(a)
    emit_stats(b)
    emit_load32(a)
    emit_qkv(a)
    emit_norm(b)
    emit_load32(b)
    emit_scores(a)
    emit_attn_out(a)
    emit_qkv(b)
    emit_scores(b)
    emit_attn_out(b)


def _build():
    if "nc" in _CACHE:
        return _CACHE["nc"]
    nc = bacc.Bacc("TRN2", target_bir_lowering=False, debug=False, num_devices=NCORES)
    io = {}
    io["x"] = nc.dram_tensor("x", [BPC, C, HW], F32, kind="ExternalInput").ap()
    io["x16"] = nc.dram_tensor("x16", [BPC, C, HW], DT, kind="ExternalInput").ap()
    for wname in ("wqt", "wkt", "wvt", "wot"):
        io[wname] = nc.dram_tensor(wname, [C, C], DT, kind="ExternalInput").ap()
    for bname in ("bq_c", "bk_c", "bo_c", "gn_s", "gn_b"):
        io[bname] = nc.dram_tensor(bname, [P, NCT], F32, kind="ExternalInput").ap()
    io["bv_r"] = nc.dram_tensor("bv_r", [1, C], F32, kind="ExternalInput").ap()
    io["gsel"] = nc.dram_tensor("gsel", [P, GPT], F32, kind="ExternalInput").ap()
    io["gselT"] = nc.dram_tensor("gselT", [GPT, P], F32, kind="ExternalInput").ap()
    io["out"] = nc.dram_tensor("out", [BPC, C, HW], F32, kind="ExternalOutput").ap()

    with tile.TileContext(nc) as tc:
        with ExitStack() as ctx:
            _emit(ctx, tc, io)
    nc.compile()
    _CACHE["nc"] = nc
    return nc


def _col_layout(v):
    # (C,) -> (P, NCT): column ct holds channels [ct*128, (ct+1)*128)
    return np.ascontiguousarray(np.asarray(v, np.float32).reshape(NCT, P).T)


def _run(inputs, trace=False, **run_kwargs):
    x = np.ascontiguousarray(np.asarray(inputs["x"], np.float32).reshape(B, C, HW))
    x16 = x.astype(DT_NP)
    wdt = {n: np.ascontiguousarray(np.asarray(inputs[s], np.float32).T).astype(DT_NP)
           for n, s in (("wqt", "wq"), ("wkt", "wk"), ("wvt", "wv"), ("wot", "wo"))}
    pidx = np.arange(P)
    gsel = (pidx[:, None] // CPG == np.arange(GPT)[None, :]).astype(np.float32)
    common = {
        **wdt,
        "bq_c": _col_layout(inputs["bq"]),
        "bk_c": _col_layout(inputs["bk"]),
        "bo_c": _col_layout(inputs["bo"]),
        "gn_s": _col_layout(inputs["gn_scale"]),
        "gn_b": _col_layout(inputs["gn_bias"]),
        "bv_r": np.ascontiguousarray(np.asarray(inputs["bv"], np.float32).reshape(1, C)),
        "gsel": gsel,
        "gselT": np.ascontiguousarray(gsel.T),
    }
    in_maps = [{"x": np.ascontiguousarray(x[m * BPC:(m + 1) * BPC]),
                "x16": np.ascontiguousarray(x16[m * BPC:(m + 1) * BPC]), **common}
               for m in range(NCORES)]
    nc = _build()
    res = run_bass_kernel_spmd(nc, in_maps, core_ids=list(range(NCORES)),
                               trace=trace, **run_kwargs)
    out = np.concatenate([r["out"] for r in res.results], axis=0)
    return out.reshape(B, C, H, W).astype(np.float32), res


def kernel(**inputs):
    out, _ = _run(inputs)
    return out
